# revision 63
# baseline (speedup 1.0000x reference)
"""Trainium2 Bass kernel for a small autoregressive transformer block with
local-windowed causal attention and a large (16k) vocab head.

Data-parallel over batch: batch item b runs on NeuronCore b (8 cores).
Per core:
  h   = embed_tab[x] + pos                      [1024, 512]  (host-prepped bf16)
  q/k/v = h @ Wq/k/v (+b)                       [1024, 512]
  s   = q @ k^T / sqrt(D) + local_causal_mask   (banded, window <= 298)
  o   = softmax(s) @ v @ Wo (+bo)
  h1  = LN(h + o);  f = relu(h1@W1+b1)@W2+b2;  h2 = LN(h1 + f)
  out = h2 @ Wh (+bh)                           [1024, 16384]

All dense projections (q/k/v, Wo, W1, W2, Wh) run as fp8e4m3 DoubleRow
matmuls with a 3-term error-compensated split
    out = A_hi @ (W_hi + W_lo) + A_lo @ W_hi
where X_hi = fp8(X) and X_lo = fp8(X - X_hi) at a shared power-of-2 scale
(activations x16 on device, weights x512 on host), so both first-order
quantization errors cancel and everything accumulates in one fp32 PSUM
group descaled by 1/8192 at eviction.  Scores/attn*v stay bf16/f32r.
Host-side prep: embedding gather + positional add (bf16), weight hi/lo
fp8 splits, mask tiles, identities.  Output is written bf16.
kernel(**inputs) takes full unsharded inputs, returns [8, 1024, 16384] f32.
"""

import math
import numpy as np

import concourse.bass as bass
import concourse.mybir as mybir
import concourse.tile as tile
from concourse import bacc
from concourse.bass_utils import run_bass_kernel_spmd

# ---- problem constants (hardcoded per contract) ----
GH = 32
GW = 32
SEQ = 1024
WIN = 9
D = 512
DFF = 1024
VOCAB = 16384
EPS = 1e-5
NEG = -1e30

P = 128
NT = SEQ // P        # 8 token chunks
DC = D // P          # 4 d chunks
FC = DFF // P        # 8 dff chunks
NV = VOCAB // 512    # 32 vocab chunks
INV_SQRT_D = 1.0 / math.sqrt(D)

F32 = mybir.dt.float32
F32R = mybir.dt.float32r
BF16 = mybir.dt.bfloat16
F8 = mybir.dt.float8e4
OUT_BF16 = True
AF = mybir.ActivationFunctionType
DR = mybir.MatmulPerfMode.DoubleRow

# fp8 scales: activations are pre-scaled by HS at fp8-quantization time
# (folded into the producing op); weights are pre-scaled by WS host-side.
# PSUM then accumulates HS*WS*(true product); evictions descale by OSCALE.
HS = 16.0
WS = 512.0
OSCALE = 1.0 / (HS * WS)


def _window_start(i: int) -> int:
    # k-window [ws, ws+512) covers all allowed keys for query chunk i
    # (max lookback is WIN*GW + WIN = 297 < 384).
    return 128 * max(0, i - 3)


def _mask_tiles() -> np.ndarray:
    idx = np.arange(SEQ)
    r, c = idx // GW, idx % GW
    allow = (
        (np.abs(r[:, None] - r[None, :]) <= WIN)
        & (np.abs(c[:, None] - c[None, :]) <= WIN)
        & (idx[None, :] <= idx[:, None])
    )
    maskf = np.where(allow, 0.0, NEG).astype(np.float32)
    tiles = np.empty((NT, P, 512), np.float32)
    for i in range(NT):
        ws = _window_start(i)
        tiles[i] = maskf[i * P : (i + 1) * P, ws : ws + 512]
    return tiles


def _r(ap):
    """bitcast to float32r for full-rate fp32 matmul."""
    return ap.bitcast(F32R)


def _bcast_ap(a: bass.AP) -> bass.AP:
    """[n] DRAM vector AP -> [P, n] partition-broadcast DMA source."""
    return bass.AP(tensor=a.tensor, offset=a.offset, ap=[[0, P], *a.ap])


def _build_program(flags: dict, wh_bufs: int = 6, msk_bufs: int = 6, lean: bool = False) -> bass.Bass:
    nc = bacc.Bacc("TRN2", target_bir_lowering=False)

    # ---------- I/O ----------
    identb_d = nc.declare_dram_parameter("identb", [P, P], BF16, False)
    hx_d = nc.declare_dram_parameter("hx", [SEQ, D], BF16, False)
    msk_d = nc.declare_dram_parameter("maskt", [NT, P, 512], BF16, False)
    wq_d = nc.declare_dram_parameter("wq", [2, D, D], F8, False)
    wk_d = nc.declare_dram_parameter("wk", [2, D, D], F8, False)
    wv_d = nc.declare_dram_parameter("wv", [2, D, D], F8, False)
    wo_d = nc.declare_dram_parameter("wo", [2, D, D], F8, False)
    w1_d = nc.declare_dram_parameter("w1", [2, D, DFF], F8, False)
    w2_d = nc.declare_dram_parameter("w2", [2, DFF, D], F8, False)
    wh_d = nc.declare_dram_parameter("wh", [D, VOCAB], F8, False)
    whb_d = nc.declare_dram_parameter("whb", [D, VOCAB], F8, False)
    dp = lambda name, shape: nc.declare_dram_parameter(name, shape, F32, False)
    bq_d = dp("bq", [D]) if flags["bq"] else None
    bk_d = dp("bk", [D]) if flags["bk"] else None
    bv_d = dp("bv", [D]) if flags["bv"] else None
    bo_d = dp("bo", [D]) if flags["bo"] else None
    b1_d = dp("b1", [DFF]) if flags["b1"] else None
    b2_d = dp("b2", [D]) if flags["b2"] else None
    bh_d = dp("bh", [VOCAB]) if flags["bh"] else None
    g1_d = dp("g1", [D]) if flags["g1"] else None
    be1_d = dp("be1", [D]) if flags["be1"] else None
    g2_d = dp("g2", [D]) if flags["g2"] else None
    be2_d = dp("be2", [D]) if flags["be2"] else None
    out_d = nc.declare_dram_parameter("out", [SEQ, VOCAB], BF16 if OUT_BF16 else F32, True)

    with tile.TileContext(nc) as tc:
        # ----- whole-kernel pools -----
        const = tc.alloc_tile_pool(name="const", bufs=1)
        small = tc.alloc_tile_pool(name="small", bufs=8)
        psum = tc.alloc_tile_pool(name="psA", bufs=4, space="PSUM")
        psum_t = tc.alloc_tile_pool(name="psT", bufs=4, space="PSUM")
        opool = tc.alloc_tile_pool(name="outev", bufs=4, side="right")
        p_h2T = tc.alloc_tile_pool(name="h2Tp", bufs=1, side="right")

        identb = const.tile([P, P], BF16, tag="identb")
        eps_t = const.tile([P, 1], F32, tag="eps")
        nc.vector.memset(eps_t[:], EPS)
        eps_hs = const.tile([P, 1], F32, tag="eps_hs")
        nc.vector.memset(eps_hs[:], EPS / (HS * HS))

        def load_col_bias(handle, nchunks, tag):
            # [nchunks*P] DRAM -> [P, nchunks] (chunk m in column m)
            t = const.tile([P, nchunks], F32, tag=tag)
            nc.sync.dma_start(out=t[:], in_=handle[:].rearrange("(m p) -> p m", p=P))
            return t

        def load_bcast(handle, n, tag):
            t = const.tile([P, n], F32, tag=tag)
            nc.sync.dma_start(out=t[:], in_=_bcast_ap(handle[:]))
            return t

        bq_sb = load_col_bias(bq_d, DC, "bq") if bq_d else None
        bk_sb = load_col_bias(bk_d, DC, "bk") if bk_d else None
        b1_sb = load_col_bias(b1_d, FC, "b1") if b1_d else None
        bv_bc = load_bcast(bv_d, D, "bv") if bv_d else None
        bo_bc = load_bcast(bo_d, D, "bo") if bo_d else None
        b2_bc = load_bcast(b2_d, D, "b2") if b2_d else None
        g1_bc = load_bcast(g1_d, D, "g1") if g1_d else None
        be1_bc = load_bcast(be1_d, D, "be1") if be1_d else None
        g2_bc = load_bcast(g2_d, D, "g2") if g2_d else None
        be2_bc = load_bcast(be2_d, D, "be2") if be2_d else None

        h8T = [p_h2T.tile([P, DC, P], F8, tag=f"h8T{j}", name=f"h8T{j}") for j in range(NT)]
        h8bT = [p_h2T.tile([P, DC, P], F8, tag=f"h8bT{j}", name=f"h8bT{j}") for j in range(NT)]

        # ----- phase A pools (left, LIFO) -----
        p_woh = tc.alloc_tile_pool(name="woh", bufs=1)         # wo, h  (-> stage 4)
        wo_sb = p_woh.tile([P, 2, DC, D], F8, tag="wo")
        h_sb = p_woh.tile([P, NT, D], BF16, tag="h")

        p_oT = tc.alloc_tile_pool(name="oTp", bufs=1)          # oT    (-> stage 4)
        oT8 = p_oT.tile([P, DC, SEQ], F8, tag="oT8")
        oT8b = p_oT.tile([P, DC, SEQ], F8, tag="oT8b")

        p_v = tc.alloc_tile_pool(name="vp", bufs=1)            # v (-> wave 2)
        v_sb = p_v.tile([P, NT, D], BF16, tag="v")
        p_at = tc.alloc_tile_pool(name="attnw", bufs=3)        # softmax work (-> stage 4)
        p_qk = tc.alloc_tile_pool(name="qkp", bufs=1)          # qT,kT (-> wave 1)
        qT = p_qk.tile([P, DC, SEQ], F32R, tag="qT")
        kT = p_qk.tile([P, DC, SEQ], F32R, tag="kT")

        p_wq = tc.alloc_tile_pool(name="wqp", bufs=1)          # wq,wk,wv,hT (-> stage 2)
        wq_sb = p_wq.tile([P, 2, DC, D], F8, tag="wq")
        wk_sb = p_wq.tile([P, 2, DC, D], F8, tag="wk")
        wv_sb = p_wq.tile([P, 2, DC, D], F8, tag="wv")
        hT8 = p_wq.tile([P, DC, SEQ], F8, tag="hT8")
        hT8b = p_wq.tile([P, DC, SEQ], F8, tag="hT8b")
        msk_all = p_wq.tile([P, NT, 512], BF16, tag="msk")

        # ---------- stage 1: h load (host pre-gathered) + transpose ----------
        # sync-queue order tuned to consumer time: identities + hx halves
        # first (stage-1 transposes), then q/k weights (stage 2), masks, v/o
        hx_r = hx_d[:].rearrange("(j p) d -> p j d", p=P)
        nc.sync.dma_start(out=identb[:], in_=identb_d[:])
        nc.sync.dma_start(out=h_sb[:, 0:4, :], in_=hx_r[:, 0:4, :])
        nc.sync.dma_start(out=wq_sb[:], in_=wq_d[:].rearrange("s (k p) o -> p s k o", p=P))
        nc.sync.dma_start(out=h_sb[:, 4:8, :], in_=hx_r[:, 4:8, :])
        nc.sync.dma_start(out=wk_sb[:], in_=wk_d[:].rearrange("s (k p) o -> p s k o", p=P))
        nc.sync.dma_start(out=msk_all[:, 0:4, :], in_=msk_d[0:4].rearrange("j p w -> p j w"))
        nc.sync.dma_start(out=wv_sb[:], in_=wv_d[:].rearrange("s (k p) o -> p s k o", p=P))
        nc.sync.dma_start(out=msk_all[:, 4:8, :], in_=msk_d[4:8].rearrange("j p w -> p j w"))
        nc.sync.dma_start(out=wo_sb[:], in_=wo_d[:].rearrange("s (k p) o -> p s k o", p=P))

        def s1_trans(j):
            pt = psum_t.tile([P, 512], BF16, tag="ptb", name=f"s1pt{j}")
            for m in range(DC):
                nc.tensor.transpose(
                    out=pt[:, m * P : (m + 1) * P],
                    in_=h_sb[:, j, m * P : (m + 1) * P],
                    identity=identb[:],
                )
            hslc = hT8[:, :, j * P : (j + 1) * P]
            nc.scalar.activation(out=hslc, in_=pt[:], func=AF.Copy, bias=0.0, scale=HS)
            nc.vector.scalar_tensor_tensor(
                out=hT8b[:, :, j * P : (j + 1) * P], in0=pt[:], scalar=HS, in1=hslc,
                op0=mybir.AluOpType.mult, op1=mybir.AluOpType.subtract,
            )

        for k in range(NT):
            s1_trans(k)

        # ---------- stage 2: qT / kT (d-major), v (token-major) ----------
        # 3-term fp8 split: hi@whi + hi@wlo + lo@whi, each over 2 DoubleRow
        # k-pairs, all accumulating at scale HS*WS in one PSUM group.
        TERMS = ((0, 0), (0, 1), (1, 0))

        # ---------- stage 3 wave 1 bodies (interleaved into stage 2) ----------
        attns = [None] * NT
        recips = [None] * NT

        def scores_i(i):
            ws = _window_start(i)
            nw = min(512, max(256, (i + 1) * P))  # live window (>=256 keeps f32r fast)
            ps_s = psum.tile([P, 512], F32, tag="ps", name=f"pss{i}")
            for ki in range(DC):
                nc.tensor.matmul(
                    ps_s[:, :nw],
                    _r(qT[:, ki, i * P : (i + 1) * P]),
                    _r(kT[:, ki, ws : ws + nw]),
                    start=(ki == 0),
                    stop=(ki == DC - 1),
                )
            s_t = p_at.tile([P, 512], F32, tag="s_t", bufs=3)
            nc.vector.tensor_add(out=s_t[:, :nw], in0=ps_s[:, :nw], in1=msk_all[:, i, :nw])
            attn = p_at.tile([P, 512], BF16, tag="attn", bufs=NT, name=f"attn{i}")
            denom = small.tile([P, 1], F32, tag="denom")
            nc.scalar.activation(
                out=attn[:, :nw], in_=s_t[:, :nw], func=AF.Exp,
                bias=0.0, scale=INV_SQRT_D,
                accum_out=denom[:, 0:1],
            )
            recip = small.tile([P, 1], F32, tag="recip", bufs=NT, name=f"recip{i}")
            nc.vector.reciprocal(out=recip[:], in_=denom[:])
            attns[i] = attn
            recips[i] = recip

        attnTs = [None] * NT
        o_ts = [None] * NT

        def w2_a(i):  # attn transposes + attnT eviction
            ws = _window_start(i)
            kb0 = ws // P
            nkb = min(DC, i - kb0 + 1)
            pt = psum_t.tile([P, 512], BF16, tag="ptb", name=f"atp{i}")
            for kk in range(nkb):
                nc.tensor.transpose(
                    out=pt[:, kk * P : (kk + 1) * P],
                    in_=attns[i][:, kk * P : (kk + 1) * P],
                    identity=identb[:],
                )
            attnT = p_at.tile([P, 512], BF16, tag="attnT", bufs=3, name=f"attnT{i}")
            nc.scalar.copy(out=attnT[:, : nkb * P], in_=pt[:, : nkb * P])
            attnTs[i] = attnT

        def w2_b(i):  # o matmuls + scale
            ws = _window_start(i)
            kb0 = ws // P
            nkb = min(DC, i - kb0 + 1)
            ps_o = psum.tile([P, 512], F32, tag="ps", name=f"pso{i}")
            for kk in range(nkb):
                nc.tensor.matmul(
                    ps_o[:],
                    attnTs[i][:, kk * P : (kk + 1) * P],
                    v_sb[:, kb0 + kk, :],
                    start=(kk == 0),
                    stop=(kk == nkb - 1),
                )
            o_t = p_at.tile([P, D], BF16, tag="o_t", bufs=3, name=f"o_t{i}")
            nc.vector.tensor_scalar_mul(out=o_t[:], in0=ps_o[:], scalar1=recips[i][:, 0:1])
            o_ts[i] = o_t

        def w2_c(i):  # oT transposes + fp8 split eviction
            pt2 = psum_t.tile([P, 512], BF16, tag="ptb", name=f"otp{i}")
            for m in range(DC):
                nc.tensor.transpose(
                    out=pt2[:, m * P : (m + 1) * P],
                    in_=o_ts[i][:, m * P : (m + 1) * P],
                    identity=identb[:],
                )
            oslc = oT8[:, :, i * P : (i + 1) * P]
            nc.scalar.activation(out=oslc, in_=pt2[:], func=AF.Copy, bias=0.0, scale=HS)
            nc.vector.scalar_tensor_tensor(
                out=oT8b[:, :, i * P : (i + 1) * P], in0=pt2[:], scalar=HS, in1=oslc,
                op0=mybir.AluOpType.mult, op1=mybir.AluOpType.subtract,
            )


        # t-major order: all groups needing hT[0:512] first (PE is in-order)
        for t in range(SEQ // 512):
            for (wt, bt, dst) in ((wq_sb, bq_sb, qT), (wk_sb, bk_sb, kT)):
                for m in range(DC):
                    ps = psum.tile([P, 512], F32, tag="ps")
                    i_mm = 0
                    for s_h, s_w in TERMS:
                        hsrc = hT8 if s_h == 0 else hT8b
                        for k0 in (0, 2):
                            nc.tensor.matmul(
                                ps[:],
                                wt[:, s_w, k0 : k0 + 2, m * P : (m + 1) * P],
                                hsrc[:, k0 : k0 + 2, t * 512 : (t + 1) * 512],
                                start=(i_mm == 0),
                                stop=(i_mm == 5),
                                perf_mode=DR,
                            )
                            i_mm += 1
                    dslc = dst[:, m, t * 512 : (t + 1) * 512]
                    if bt is not None:
                        nc.scalar.activation(
                            out=dslc, in_=ps[:], func=AF.Identity,
                            bias=bt[:, m : m + 1], scale=OSCALE,
                        )
                    elif dst is kT:
                        nc.vector.tensor_scalar_mul(out=dslc, in0=ps[:], scalar1=OSCALE)
                    else:
                        nc.scalar.activation(
                            out=dslc, in_=ps[:], func=AF.Copy, bias=0.0, scale=OSCALE,
                        )
            for j in range(4 * t, 4 * t + 4):
                ps = psum.tile([P, 512], F32, tag="ps")
                i_mm = 0
                for s_h, s_w in TERMS:
                    hsrc = hT8 if s_h == 0 else hT8b
                    for k0 in (0, 2):
                        nc.tensor.matmul(
                            ps[:],
                            hsrc[:, k0 : k0 + 2, j * P : (j + 1) * P],
                            wv_sb[:, s_w, k0 : k0 + 2, :],
                            start=(i_mm == 0),
                            stop=(i_mm == 5),
                            perf_mode=DR,
                        )
                        i_mm += 1
                if bv_bc is not None:
                    nc.vector.scalar_tensor_tensor(
                        out=v_sb[:, j, :], in0=ps[:], scalar=OSCALE, in1=bv_bc[:],
                        op0=mybir.AluOpType.mult, op1=mybir.AluOpType.add,
                    )
                else:
                    nc.vector.tensor_scalar_mul(out=v_sb[:, j, :], in0=ps[:], scalar1=OSCALE)
            # scores for query chunks whose full k-window is now resident;
            # second half also ramps the attention pipeline (slots 0..3)
            for i in range(4 * t, 4 * t + 4):
                scores_i(i)
                if t == 1:
                    k_ = i - 4
                    w2_a(k_)
                    if k_ >= 1:
                        w2_b(k_ - 1)
                    if k_ >= 3:
                        w2_c(k_ - 3)

        p_wq.release()
        p_qk.release()

        # ----- right-side pools for FFN phase -----
        whpool = tc.alloc_tile_pool(name="whstream", bufs=wh_bufs, side="right")
        p_h1 = tc.alloc_tile_pool(name="h1p", bufs=1, side="right")
        h1_sb = p_h1.tile([P, NT, D], BF16, tag="h1")
        h1T8 = p_h1.tile([P, DC, SEQ], F8, tag="h1T8")
        h1T8b = p_h1.tile([P, DC, SEQ], F8, tag="h1T8b")
        w1_sb = p_h1.tile([P, 2, DC, DFF], F8, tag="w1")
        nc.sync.dma_start(out=w1_sb[:], in_=w1_d[:].rearrange("s (k p) o -> p s k o", p=P))

        # ---------- stage 3 wave 2 + stage 4, software-pipelined ----------
        p_st4 = tc.alloc_tile_pool(name="st4", bufs=3)
        def s4_proj(j):  # attn projection + residual + LN1 (no transpose)
            ps = psum.tile([P, 512], F32, tag="ps", name=f"psp{j}")
            i_mm = 0
            for s_o, s_w in TERMS:
                osrc = oT8 if s_o == 0 else oT8b
                for k0 in (0, 2):
                    nc.tensor.matmul(
                        ps[:],
                        osrc[:, k0 : k0 + 2, j * P : (j + 1) * P],
                        wo_sb[:, s_w, k0 : k0 + 2, :],
                        start=(i_mm == 0),
                        stop=(i_mm == 5),
                        perf_mode=DR,
                    )
                    i_mm += 1
            r1 = p_st4.tile([P, D], F32, tag="r1", name=f"r1_{j}")
            nc.vector.scalar_tensor_tensor(
                out=r1[:], in0=ps[:], scalar=OSCALE, in1=h_sb[:, j, :],
                op0=mybir.AluOpType.mult, op1=mybir.AluOpType.add,
            )
            if bo_bc is not None:
                nc.vector.tensor_add(out=r1[:], in0=r1[:], in1=bo_bc[:])
            stats = small.tile([P, 6], F32, tag="stats")
            nc.vector.bn_stats(out=stats[:], in_=r1[:])
            mv = small.tile([P, 2], F32, tag="mv")
            nc.vector.bn_aggr(out=mv[:], in_=stats[:])
            stdt = small.tile([P, 1], F32, tag="stdt")
            nc.scalar.activation(
                out=stdt[:], in_=mv[:, 1:2], func=AF.Sqrt,
                bias=eps_t[:, 0:1], scale=1.0,
            )
            rstd = small.tile([P, 1], F32, tag="rstd")
            nc.vector.reciprocal(out=rstd[:], in_=stdt[:])
            nc.vector.tensor_scalar(
                out=h1_sb[:, j, :], in0=r1[:],
                scalar1=mv[:, 0:1], scalar2=rstd[:, 0:1],
                op0=mybir.AluOpType.subtract, op1=mybir.AluOpType.mult,
            )
            if g1_bc is not None:
                nc.vector.tensor_mul(out=h1_sb[:, j, :], in0=h1_sb[:, j, :], in1=g1_bc[:])
            if be1_bc is not None:
                nc.vector.tensor_add(out=h1_sb[:, j, :], in0=h1_sb[:, j, :], in1=be1_bc[:])

        def s4_trans(j):  # h1 transposes + fp8 split eviction
            pt3 = psum_t.tile([P, 512], BF16, tag="ptb", name=f"h1p{j}")
            for m in range(DC):
                nc.tensor.transpose(
                    out=pt3[:, m * P : (m + 1) * P],
                    in_=h1_sb[:, j, m * P : (m + 1) * P],
                    identity=identb[:],
                )
            h1slc = h1T8[:, :, j * P : (j + 1) * P]
            nc.scalar.activation(out=h1slc, in_=pt3[:], func=AF.Copy, bias=0.0, scale=HS)
            nc.vector.scalar_tensor_tensor(
                out=h1T8b[:, :, j * P : (j + 1) * P], in0=pt3[:], scalar=HS, in1=h1slc,
                op0=mybir.AluOpType.mult, op1=mybir.AluOpType.subtract,
            )

        # deeper pipeline: 2 slots of slack before the projection (oT fp8
        # quant chain) and before the h1 transpose (LN1 chain)
        p_w12 = tc.alloc_tile_pool(name="w12", bufs=1, side="right")
        w2_sb = p_w12.tile([P, 2, FC, D], F8, tag="w2")
        nc.sync.dma_start(out=w2_sb[:], in_=w2_d[:].rearrange("s (k p) o -> p s k o", p=P))

        # ---------- stage 5: FFN up, f1T = relu(W1^T @ h1T + b1) ----------
        p_f1 = tc.alloc_tile_pool(name="f1p", bufs=1, side="right")
        f1T8 = p_f1.tile([P, FC, SEQ], F8, tag="f1T8")
        f1T8b = p_f1.tile([P, FC, SEQ], F8, tag="f1T8b")
        def ffn1_group(n, t):
            ps = psum.tile([P, 512], F32, tag="ps", name=f"psf{n}_{t}")
            i_mm = 0
            for s_h, s_w in TERMS:
                hsrc = h1T8 if s_h == 0 else h1T8b
                for k0 in (0, 2):
                    nc.tensor.matmul(
                        ps[:],
                        w1_sb[:, s_w, k0 : k0 + 2, n * P : (n + 1) * P],
                        hsrc[:, k0 : k0 + 2, t * 512 : (t + 1) * 512],
                        start=(i_mm == 0),
                        stop=(i_mm == 5),
                        perf_mode=DR,
                    )
                    i_mm += 1
            # f1 = relu(psum*OSCALE + b1), stored as fp8 hi+lo at scale HS
            # (b1 is host-prescaled by HS, activation scale folds HS*OSCALE)
            ftmp = p_f1.tile([P, 512], F32, tag="ftmp", bufs=2, name=f"ftmp{n}_{t}")
            if b1_sb is not None:
                nc.scalar.activation(
                    out=ftmp[:], in_=ps[:], func=AF.Relu,
                    bias=b1_sb[:, n : n + 1], scale=HS * OSCALE,
                )
            else:
                nc.vector.tensor_scalar(
                    out=ftmp[:], in0=ps[:],
                    scalar1=0.0, scalar2=HS * OSCALE,
                    op0=mybir.AluOpType.max, op1=mybir.AluOpType.mult,
                )
            fslc = f1T8[:, n, t * 512 : (t + 1) * 512]
            nc.gpsimd.tensor_copy(out=fslc, in_=ftmp[:])
            nc.gpsimd.tensor_sub(
                out=f1T8b[:, n, t * 512 : (t + 1) * 512], in0=ftmp[:], in1=fslc,
            )


        for k in range(4, NT + 7):
            if k < NT:
                w2_a(k)
            if 1 <= k < NT + 1:
                w2_b(k - 1)
            if 3 <= k < NT + 3:
                w2_c(k - 3)
            if 5 <= k < NT + 5:
                s4_proj(k - 5)
            if 7 <= k < NT + 7:
                s4_trans(k - 7)
            if 11 <= k:
                # fill the pipeline tail with FFN-up work on the first half
                ffn1_group(2 * (k - 11), 0)
                ffn1_group(2 * (k - 11) + 1, 0)

        p_st4.release()
        p_at.release()
        p_v.release()
        p_oT.release()
        p_woh.release()

        # ---------- stage 6: FFN down + residual + LN2 (pipelined) ----------
        def s6_main(j):
            ps = psum.tile([P, 512], F32, tag="ps", name=f"ps6_{j}")
            i_mm = 0
            for s_f, s_w in TERMS:
                fsrc = f1T8 if s_f == 0 else f1T8b
                for n0 in (0, 2, 4, 6):
                    nc.tensor.matmul(
                        ps[:],
                        fsrc[:, n0 : n0 + 2, j * P : (j + 1) * P],
                        w2_sb[:, s_w, n0 : n0 + 2, :],
                        start=(i_mm == 0),
                        stop=(i_mm == 11),
                        perf_mode=DR,
                    )
                    i_mm += 1
            r2 = p_f1.tile([P, D], F32, tag="r2", bufs=3, name=f"r2_{j}")
            nc.vector.scalar_tensor_tensor(
                out=r2[:], in0=ps[:], scalar=OSCALE, in1=h1_sb[:, j, :],
                op0=mybir.AluOpType.mult, op1=mybir.AluOpType.add,
            )
            if b2_bc is not None:
                nc.vector.tensor_add(out=r2[:], in0=r2[:], in1=b2_bc[:])
            stats = small.tile([P, 6], F32, tag="stats")
            nc.vector.bn_stats(out=stats[:], in_=r2[:])
            mv = small.tile([P, 2], F32, tag="mv")
            nc.vector.bn_aggr(out=mv[:], in_=stats[:])
            stdt = small.tile([P, 1], F32, tag="stdt")
            # sqrt((var + eps)/HS^2) = sqrt(var+eps)/HS, so rstd = HS/std:
            # h2_t holds HS * LN(r2), pre-scaled for the fp8 head.
            nc.scalar.activation(
                out=stdt[:], in_=mv[:, 1:2], func=AF.Sqrt,
                bias=eps_hs[:, 0:1], scale=1.0 / (HS * HS),
            )
            rstd = small.tile([P, 1], F32, tag="rstd")
            nc.vector.reciprocal(out=rstd[:], in_=stdt[:])
            h2_t = p_f1.tile([P, D], BF16, tag="h2_t", bufs=3, name=f"h2t_{j}")
            nc.vector.tensor_scalar(
                out=h2_t[:], in0=r2[:],
                scalar1=mv[:, 0:1], scalar2=rstd[:, 0:1],
                op0=mybir.AluOpType.subtract, op1=mybir.AluOpType.mult,
            )
            if g2_bc is not None:
                nc.vector.tensor_mul(out=h2_t[:], in0=h2_t[:], in1=g2_bc[:])
            if be2_bc is not None:
                nc.vector.tensor_add(out=h2_t[:], in0=h2_t[:], in1=be2_bc[:])
            return h2_t

        h2ts = [None] * NT

        def s6_trans(j):
            pt = psum_t.tile([P, 512], BF16, tag="ptb", name=f"h2p{j}")
            for m in range(DC):
                nc.tensor.transpose(
                    out=pt[:, m * P : (m + 1) * P],
                    in_=h2ts[j][:, m * P : (m + 1) * P],
                    identity=identb[:],
                )
            # pt holds HS*h2T; quantize to fp8 + same-scale fp8 residual
            nc.scalar.copy(out=h8T[j][:, :, :], in_=pt[:])
            nc.vector.tensor_sub(out=h8bT[j][:, :, :], in0=pt[:], in1=h8T[j][:, :, :])

        # head chunks for vc=0..3 interleaved into stage-6 so PE fills LN waits
        wh_rr = wh_d[:].rearrange("(k p) v -> p k v", p=P)
        whb_rr = whb_d[:].rearrange("(k p) v -> p k v", p=P)
        _bh_tiles = {}

        def load_whv(vc, name):
            wv = whpool.tile([P, DC, 512], F8, tag="whv", name=f"whv{name}")
            nc.sync.dma_start(out=wv[:], in_=wh_rr[:, :, vc * 512 : (vc + 1) * 512])
            wvb = whpool.tile([P, DC, 512], F8, tag="whvb", name=f"whvb{name}")
            nc.sync.dma_start(out=wvb[:], in_=whb_rr[:, :, vc * 512 : (vc + 1) * 512])
            if bh_d is not None:
                bh_bc = whpool.tile([P, 512], F32, tag="bh", bufs=8, name=f"bh{name}")
                nc.sync.dma_start(
                    out=bh_bc[:], in_=_bcast_ap(bh_d[vc * 512 : (vc + 1) * 512])
                )
                _bh_tiles[vc] = bh_bc
            return wv, wvb

        N_IL = 4  # head vocab chunks interleaved into the stage-6 loop
        whvs = {}
        otiles = {}
        for vc in range(2):
            whvs[vc] = load_whv(vc, str(vc))
            otiles[vc] = opool.tile(
                [P, NT, 512], BF16 if OUT_BF16 else F32, tag="ot", name=f"otile{vc}"
            )

        def head_j(whv, otile, j, toggle):
            wv, wvb = whv
            ps = psum.tile([P, 512], F32, tag="ps", name=f"psh{toggle}_{j}")
            terms = ((h8T[j], wv), (h8T[j], wvb), (h8bT[j], wv))
            n_mm = len(terms) * 2
            i_mm = 0
            for lh, rh in terms:
                for k0 in (0, 2):
                    nc.tensor.matmul(
                        ps[:],
                        lh[:, k0 : k0 + 2, :],
                        rh[:, k0 : k0 + 2, :],
                        start=(i_mm == 0),
                        stop=(i_mm == n_mm - 1),
                        perf_mode=DR,
                    )
                    i_mm += 1
            if bh_sb_for(toggle) is not None:
                nc.vector.scalar_tensor_tensor(
                    out=otile[:, j, :], in0=ps[:], scalar=OSCALE,
                    in1=bh_sb_for(toggle)[:],
                    op0=mybir.AluOpType.mult, op1=mybir.AluOpType.add,
                )
            elif j % 2 == 0:
                nc.vector.tensor_scalar_mul(out=otile[:, j, :], in0=ps[:], scalar1=OSCALE)
            else:
                nc.scalar.activation(
                    out=otile[:, j, :], in_=ps[:], func=AF.Copy,
                    bias=0.0, scale=OSCALE,
                )

        def bh_sb_for(key):
            return _bh_tiles.get(key)

        for n in range(FC):
            ffn1_group(n, 1)
            if n % 2 == 1:
                h2ts[n // 2] = s6_main(n // 2)

        out_r = out_d[:].rearrange("(j p) v -> p j v", p=P)

        def out_dma(vc, otile, half):
            sl = slice(0, 4) if half == 0 else slice(4, 8)
            nc.sync.dma_start(
                out=out_r[:, sl, vc * 512 : (vc + 1) * 512], in_=otile[:, sl, :]
            )

        for k in range(NT + N_IL + 3):
            if 4 <= k < NT:
                h2ts[k] = s6_main(k)
            if 2 <= k <= NT + 1:
                s6_trans(k - 2)
            if k == 0:
                whvs[2] = load_whv(2, "2")
                otiles[2] = opool.tile(
                    [P, NT, 512], BF16 if OUT_BF16 else F32, tag="ot", name="otile2"
                )
            if k == 2:
                whvs[3] = load_whv(3, "3")
                otiles[3] = opool.tile(
                    [P, NT, 512], BF16 if OUT_BF16 else F32, tag="ot", name="otile3"
                )
            if k == 4:
                whvs[4] = load_whv(4, "4")
            if k == 6:
                whvs[5] = load_whv(5, "5")
            for vcix in range(N_IL):
                kk = k - 3 - vcix
                if 0 <= kk < NT:
                    head_j(whvs[vcix], otiles[vcix], kk, vcix)
                    if kk == 3:
                        out_dma(vcix, otiles[vcix], 0)
                    elif kk == NT - 1:
                        out_dma(vcix, otiles[vcix], 1)

        p_f1.release()
        p_w12.release()
        p_h1.release()

        # ---------- stage 7: vocab head (vc >= N_IL) ----------
        for vc in range(N_IL, NV):
            whv = whvs.get(vc) or load_whv(vc, str(vc))
            otile = opool.tile([P, NT, 512], BF16 if OUT_BF16 else F32, tag="ot")
            last = vc >= NV - 2
            for j in range(NT):
                head_j(whv, otile, j, vc)
                if j == 3:
                    out_dma(vc, otile, 0)
                elif last and j >= 4:
                    # last chunk: per-j tail so the final DMA is tiny
                    nc.sync.dma_start(
                        out=out_r[:, j, vc * 512 : (vc + 1) * 512],
                        in_=otile[:, j, :],
                    )
            if not last:
                out_dma(vc, otile, 1)

        whpool.release()
        p_h2T.release()
        opool.release()
        psum_t.release()
        psum.release()
        small.release()
        const.release()

    nc.finalize()
    return nc


_PROGRAM_CACHE: dict = {}


def _get_program(flags: dict) -> bass.Bass:
    key = tuple(sorted(flags.items()))
    if key not in _PROGRAM_CACHE:
        _PROGRAM_CACHE[key] = _build_program(flags)
    return _PROGRAM_CACHE[key]


def _prep(x, embed_tab, row_embed, col_embed, Wq, bq, Wk, bk, Wv, bv, Wo, bo,
          ln1_g, ln1_b, W1, b1, W2, b2, ln2_g, ln2_b, Wh, bh):
    """Shared host-side prep: flags, common input map, per-core x shards."""
    f32c = lambda a: np.ascontiguousarray(np.asarray(a, dtype=np.float32))
    x = np.asarray(x)
    B = x.shape[0]
    assert x.shape == (B, SEQ)

    import ml_dtypes
    bf16c = lambda a: np.ascontiguousarray(
        np.asarray(a, dtype=np.float32).astype(ml_dtypes.bfloat16)
    )

    def split8(w):
        ws = f32c(w) * np.float32(WS)
        hi = ws.astype(ml_dtypes.float8_e4m3)
        lo = (ws - hi.astype(np.float32)).astype(ml_dtypes.float8_e4m3)
        return np.ascontiguousarray(np.stack([hi, lo], axis=0))

    arrs = dict(
        wq=split8(Wq), wk=split8(Wk), wv=split8(Wv),
        wo=split8(Wo), w1=split8(W1), w2=split8(W2),
        identb=np.eye(P, dtype=np.float32).astype(ml_dtypes.bfloat16),
    )
    w512 = f32c(Wh) * np.float32(WS)
    w8 = w512.astype(ml_dtypes.float8_e4m3)
    w8b = (w512 - w8.astype(np.float32)).astype(ml_dtypes.float8_e4m3)
    arrs["wh"] = np.ascontiguousarray(w8)
    arrs["whb"] = np.ascontiguousarray(w8b)
    # input prep: embedding rows gathered + positional encoding added on host,
    # shipped per-core as a dense bf16 [SEQ, D] activation
    pos = np.concatenate(
        [np.repeat(f32c(row_embed), GW, axis=0), np.tile(f32c(col_embed), (GH, 1))],
        axis=-1,
    ).astype(np.float32)
    emb32 = f32c(embed_tab)
    hxs = [bf16c(emb32[x[c]] + pos) for c in range(B)]
    arrs["maskt"] = _mask_tiles().astype(ml_dtypes.bfloat16)

    # b1 feeds the HS-prescaled relu; be2 the HS-prescaled LN2 output
    bias_map = dict(
        bq=f32c(bq), bk=f32c(bk), bv=f32c(bv), bo=f32c(bo),
        b1=f32c(b1) * np.float32(HS),
        b2=f32c(b2), bh=f32c(bh), be1=f32c(ln1_b), be2=f32c(ln2_b) * np.float32(HS),
    )
    gain_map = dict(g1=f32c(ln1_g), g2=f32c(ln2_g))
    flags = {k: bool(np.any(v)) for k, v in bias_map.items()}
    flags.update({k: bool(np.any(v != 1.0)) for k, v in gain_map.items()})
    for k, v in {**bias_map, **gain_map}.items():
        if flags[k]:
            arrs[k] = v

    return flags, arrs, hxs, B


def kernel(**inputs):
    flags, arrs, hxs, B = _prep(**inputs)
    nc = _get_program(flags)
    core_ids = list(range(8))
    in_maps = [{**arrs, "hx": hxs[c % B]} for c in core_ids]
    res = run_bass_kernel_spmd(nc, in_maps, core_ids)
    out = np.stack([res.results[c]["out"] for c in range(B)], axis=0)
    return np.asarray(out, dtype=np.float32)



# revision 64
# speedup vs baseline: 1.0424x; 1.0424x over previous
"""Trainium2 Bass kernel for a small autoregressive transformer block with
local-windowed causal attention and a large (16k) vocab head.

Data-parallel over batch: batch item b runs on NeuronCore b (8 cores).
Per core:
  h   = embed_tab[x] + pos                      [1024, 512]  (host-prepped bf16)
  q/k/v = h @ Wq/k/v (+b)                       [1024, 512]
  s   = q @ k^T / sqrt(D) + local_causal_mask   (banded, window <= 298)
  o   = softmax(s) @ v @ Wo (+bo)
  h1  = LN(h + o);  f = relu(h1@W1+b1)@W2+b2;  h2 = LN(h1 + f)
  out = h2 @ Wh (+bh)                           [1024, 16384]

All dense projections (q/k/v, Wo, W1, W2, Wh) run as fp8e4m3 DoubleRow
matmuls with a 3-term error-compensated split
    out = A_hi @ (W_hi + W_lo) + A_lo @ W_hi
where X_hi = fp8(X) and X_lo = fp8(X - X_hi) at a shared power-of-2 scale
(activations x16 on device, weights x512 on host), so both first-order
quantization errors cancel and everything accumulates in one fp32 PSUM
group descaled by 1/8192 at eviction.  Scores/attn*v stay bf16/f32r.
Host-side prep: embedding gather + positional add (bf16), weight hi/lo
fp8 splits, mask tiles, identities.  Output is written bf16.
kernel(**inputs) takes full unsharded inputs, returns [8, 1024, 16384] f32.
"""

import math
import numpy as np

import concourse.bass as bass
import concourse.mybir as mybir
import concourse.tile as tile
from concourse import bacc
from concourse.bass_utils import run_bass_kernel_spmd

# ---- problem constants (hardcoded per contract) ----
GH = 32
GW = 32
SEQ = 1024
WIN = 9
D = 512
DFF = 1024
VOCAB = 16384
EPS = 1e-5
NEG = -1e30

P = 128
NT = SEQ // P        # 8 token chunks
DC = D // P          # 4 d chunks
FC = DFF // P        # 8 dff chunks
NV = VOCAB // 512    # 32 vocab chunks
INV_SQRT_D = 1.0 / math.sqrt(D)

F32 = mybir.dt.float32
F32R = mybir.dt.float32r
BF16 = mybir.dt.bfloat16
F8 = mybir.dt.float8e4
OUT_BF16 = True
AF = mybir.ActivationFunctionType
DR = mybir.MatmulPerfMode.DoubleRow

# fp8 scales: activations are pre-scaled by HS at fp8-quantization time
# (folded into the producing op); weights are pre-scaled by WS host-side.
# PSUM then accumulates HS*WS*(true product); evictions descale by OSCALE.
HS = 16.0
WS = 512.0
OSCALE = 1.0 / (HS * WS)


def _window_start(i: int) -> int:
    # k-window [ws, ws+512) covers all allowed keys for query chunk i
    # (max lookback is WIN*GW + WIN = 297 < 384).
    return 128 * max(0, i - 3)


def _mask_tiles() -> np.ndarray:
    idx = np.arange(SEQ)
    r, c = idx // GW, idx % GW
    allow = (
        (np.abs(r[:, None] - r[None, :]) <= WIN)
        & (np.abs(c[:, None] - c[None, :]) <= WIN)
        & (idx[None, :] <= idx[:, None])
    )
    maskf = np.where(allow, 0.0, NEG).astype(np.float32)
    tiles = np.empty((NT, P, 512), np.float32)
    for i in range(NT):
        ws = _window_start(i)
        tiles[i] = maskf[i * P : (i + 1) * P, ws : ws + 512]
    return tiles


def _r(ap):
    """bitcast to float32r for full-rate fp32 matmul."""
    return ap.bitcast(F32R)


def _bcast_ap(a: bass.AP) -> bass.AP:
    """[n] DRAM vector AP -> [P, n] partition-broadcast DMA source."""
    return bass.AP(tensor=a.tensor, offset=a.offset, ap=[[0, P], *a.ap])


def _build_program(flags: dict, wh_bufs: int = 6, msk_bufs: int = 6, lean: bool = False) -> bass.Bass:
    nc = bacc.Bacc("TRN2", target_bir_lowering=False)

    # ---------- I/O ----------
    identb_d = nc.declare_dram_parameter("identb", [P, P], BF16, False)
    hx_d = nc.declare_dram_parameter("hx", [SEQ, D], BF16, False)
    msk_d = nc.declare_dram_parameter("maskt", [NT, P, 512], BF16, False)
    wq_d = nc.declare_dram_parameter("wq", [2, D, D], F8, False)
    wk_d = nc.declare_dram_parameter("wk", [2, D, D], F8, False)
    wv_d = nc.declare_dram_parameter("wv", [2, D, D], F8, False)
    wo_d = nc.declare_dram_parameter("wo", [2, D, D], F8, False)
    w1_d = nc.declare_dram_parameter("w1", [2, D, DFF], F8, False)
    w2_d = nc.declare_dram_parameter("w2", [2, DFF, D], F8, False)
    wh_d = nc.declare_dram_parameter("wh", [D, VOCAB], F8, False)
    whb_d = nc.declare_dram_parameter("whb", [D, VOCAB], F8, False)
    dp = lambda name, shape: nc.declare_dram_parameter(name, shape, F32, False)
    bq_d = dp("bq", [D]) if flags["bq"] else None
    bk_d = dp("bk", [D]) if flags["bk"] else None
    bv_d = dp("bv", [D]) if flags["bv"] else None
    bo_d = dp("bo", [D]) if flags["bo"] else None
    b1_d = dp("b1", [DFF]) if flags["b1"] else None
    b2_d = dp("b2", [D]) if flags["b2"] else None
    bh_d = dp("bh", [VOCAB]) if flags["bh"] else None
    g1_d = dp("g1", [D]) if flags["g1"] else None
    be1_d = dp("be1", [D]) if flags["be1"] else None
    g2_d = dp("g2", [D]) if flags["g2"] else None
    be2_d = dp("be2", [D]) if flags["be2"] else None
    out_d = nc.declare_dram_parameter("out", [SEQ, VOCAB], BF16 if OUT_BF16 else F32, True)

    with tile.TileContext(nc) as tc:
        # ----- whole-kernel pools -----
        const = tc.alloc_tile_pool(name="const", bufs=1)
        small = tc.alloc_tile_pool(name="small", bufs=8)
        psum = tc.alloc_tile_pool(name="psA", bufs=4, space="PSUM")
        psum_t = tc.alloc_tile_pool(name="psT", bufs=4, space="PSUM")
        opool = tc.alloc_tile_pool(name="outev", bufs=4, side="right")
        p_h2T = tc.alloc_tile_pool(name="h2Tp", bufs=1, side="right")

        identb = const.tile([P, P], BF16, tag="identb")
        eps_t = const.tile([P, 1], F32, tag="eps")
        nc.vector.memset(eps_t[:], EPS)
        eps_hs = const.tile([P, 1], F32, tag="eps_hs")
        nc.vector.memset(eps_hs[:], EPS / (HS * HS))

        def load_col_bias(handle, nchunks, tag):
            # [nchunks*P] DRAM -> [P, nchunks] (chunk m in column m)
            t = const.tile([P, nchunks], F32, tag=tag)
            nc.sync.dma_start(out=t[:], in_=handle[:].rearrange("(m p) -> p m", p=P))
            return t

        def load_bcast(handle, n, tag):
            t = const.tile([P, n], F32, tag=tag)
            nc.sync.dma_start(out=t[:], in_=_bcast_ap(handle[:]))
            return t

        bq_sb = load_col_bias(bq_d, DC, "bq") if bq_d else None
        bk_sb = load_col_bias(bk_d, DC, "bk") if bk_d else None
        b1_sb = load_col_bias(b1_d, FC, "b1") if b1_d else None
        bv_bc = load_bcast(bv_d, D, "bv") if bv_d else None
        bo_bc = load_bcast(bo_d, D, "bo") if bo_d else None
        b2_bc = load_bcast(b2_d, D, "b2") if b2_d else None
        g1_bc = load_bcast(g1_d, D, "g1") if g1_d else None
        be1_bc = load_bcast(be1_d, D, "be1") if be1_d else None
        g2_bc = load_bcast(g2_d, D, "g2") if g2_d else None
        be2_bc = load_bcast(be2_d, D, "be2") if be2_d else None

        h8T = [p_h2T.tile([P, DC, P], F8, tag=f"h8T{j}", name=f"h8T{j}") for j in range(NT)]
        h8bT = [p_h2T.tile([P, DC, P], F8, tag=f"h8bT{j}", name=f"h8bT{j}") for j in range(NT)]

        # ----- phase A pools (left, LIFO) -----
        p_woh = tc.alloc_tile_pool(name="woh", bufs=1)         # wo, h  (-> stage 4)
        wo_sb = p_woh.tile([P, 2, DC, D], F8, tag="wo")
        h_sb = p_woh.tile([P, NT, D], BF16, tag="h")

        p_oT = tc.alloc_tile_pool(name="oTp", bufs=1)          # oT    (-> stage 4)
        oT8 = p_oT.tile([P, DC, SEQ], F8, tag="oT8")
        oT8b = p_oT.tile([P, DC, SEQ], F8, tag="oT8b")

        p_v = tc.alloc_tile_pool(name="vp", bufs=1)            # v (-> wave 2)
        v_sb = p_v.tile([P, NT, D], BF16, tag="v")
        p_at = tc.alloc_tile_pool(name="attnw", bufs=3)        # softmax work (-> stage 4)
        p_qk = tc.alloc_tile_pool(name="qkp", bufs=1)          # qT,kT (-> wave 1)
        qT = p_qk.tile([P, DC, SEQ], F32R, tag="qT")
        kT = p_qk.tile([P, DC, SEQ], F32R, tag="kT")

        p_wq = tc.alloc_tile_pool(name="wqp", bufs=1)          # wq,wk,wv,hT (-> stage 2)
        wq_sb = p_wq.tile([P, 2, DC, D], F8, tag="wq")
        wk_sb = p_wq.tile([P, 2, DC, D], F8, tag="wk")
        wv_sb = p_wq.tile([P, 2, DC, D], F8, tag="wv")
        hT8 = p_wq.tile([P, DC, SEQ], F8, tag="hT8")
        hT8b = p_wq.tile([P, DC, SEQ], F8, tag="hT8b")
        msk_all = p_wq.tile([P, NT, 512], BF16, tag="msk")

        # ---------- stage 1: h load (host pre-gathered) + transpose ----------
        # sync-queue order tuned to consumer time: identities + hx halves
        # first (stage-1 transposes), then q/k weights (stage 2), masks, v/o
        hx_r = hx_d[:].rearrange("(j p) d -> p j d", p=P)
        nc.sync.dma_start(out=identb[:], in_=identb_d[:])
        nc.sync.dma_start(out=h_sb[:, 0:4, :], in_=hx_r[:, 0:4, :])
        nc.sync.dma_start(out=wq_sb[:], in_=wq_d[:].rearrange("s (k p) o -> p s k o", p=P))
        nc.sync.dma_start(out=h_sb[:, 4:8, :], in_=hx_r[:, 4:8, :])
        nc.sync.dma_start(out=wk_sb[:], in_=wk_d[:].rearrange("s (k p) o -> p s k o", p=P))
        nc.sync.dma_start(out=msk_all[:, 0:4, :], in_=msk_d[0:4].rearrange("j p w -> p j w"))
        nc.sync.dma_start(out=wv_sb[:], in_=wv_d[:].rearrange("s (k p) o -> p s k o", p=P))
        nc.sync.dma_start(out=msk_all[:, 4:8, :], in_=msk_d[4:8].rearrange("j p w -> p j w"))
        nc.sync.dma_start(out=wo_sb[:], in_=wo_d[:].rearrange("s (k p) o -> p s k o", p=P))

        def s1_trans(j):
            pt = psum_t.tile([P, 512], BF16, tag="ptb", name=f"s1pt{j}")
            for m in range(DC):
                nc.tensor.transpose(
                    out=pt[:, m * P : (m + 1) * P],
                    in_=h_sb[:, j, m * P : (m + 1) * P],
                    identity=identb[:],
                )
            hslc = hT8[:, :, j * P : (j + 1) * P]
            nc.scalar.activation(out=hslc, in_=pt[:], func=AF.Copy, bias=0.0, scale=HS)
            nc.vector.scalar_tensor_tensor(
                out=hT8b[:, :, j * P : (j + 1) * P], in0=pt[:], scalar=HS, in1=hslc,
                op0=mybir.AluOpType.mult, op1=mybir.AluOpType.subtract,
            )

        for k in range(NT):
            s1_trans(k)

        # ---------- stage 2: qT / kT (d-major), v (token-major) ----------
        # 3-term fp8 split: hi@whi + hi@wlo + lo@whi, each over 2 DoubleRow
        # k-pairs, all accumulating at scale HS*WS in one PSUM group.
        TERMS = ((0, 0), (0, 1), (1, 0))

        # ---------- stage 3 wave 1 bodies (interleaved into stage 2) ----------
        attns = [None] * NT
        recips = [None] * NT

        def scores_i(i):
            ws = _window_start(i)
            nw = min(512, max(256, (i + 1) * P))  # live window (>=256 keeps f32r fast)
            ps_s = psum.tile([P, 512], F32, tag="ps", name=f"pss{i}")
            for ki in range(DC):
                nc.tensor.matmul(
                    ps_s[:, :nw],
                    _r(qT[:, ki, i * P : (i + 1) * P]),
                    _r(kT[:, ki, ws : ws + nw]),
                    start=(ki == 0),
                    stop=(ki == DC - 1),
                )
            s_t = p_at.tile([P, 512], F32, tag="s_t", bufs=3)
            nc.vector.tensor_add(out=s_t[:, :nw], in0=ps_s[:, :nw], in1=msk_all[:, i, :nw])
            attn = p_at.tile([P, 512], BF16, tag="attn", bufs=NT, name=f"attn{i}")
            denom = small.tile([P, 1], F32, tag="denom")
            nc.scalar.activation(
                out=attn[:, :nw], in_=s_t[:, :nw], func=AF.Exp,
                bias=0.0, scale=INV_SQRT_D,
                accum_out=denom[:, 0:1],
            )
            recip = small.tile([P, 1], F32, tag="recip", bufs=NT, name=f"recip{i}")
            nc.vector.reciprocal(out=recip[:], in_=denom[:])
            attns[i] = attn
            recips[i] = recip

        attnTs = [None] * NT
        o_ts = [None] * NT

        def w2_a(i):  # attn transposes + attnT eviction
            ws = _window_start(i)
            kb0 = ws // P
            nkb = min(DC, i - kb0 + 1)
            pt = psum_t.tile([P, 512], BF16, tag="ptb", name=f"atp{i}")
            for kk in range(nkb):
                nc.tensor.transpose(
                    out=pt[:, kk * P : (kk + 1) * P],
                    in_=attns[i][:, kk * P : (kk + 1) * P],
                    identity=identb[:],
                )
            attnT = p_at.tile([P, 512], BF16, tag="attnT", bufs=3, name=f"attnT{i}")
            nc.scalar.copy(out=attnT[:, : nkb * P], in_=pt[:, : nkb * P])
            attnTs[i] = attnT

        def w2_b(i):  # o matmuls + scale
            ws = _window_start(i)
            kb0 = ws // P
            nkb = min(DC, i - kb0 + 1)
            ps_o = psum.tile([P, 512], F32, tag="ps", name=f"pso{i}")
            for kk in range(nkb):
                nc.tensor.matmul(
                    ps_o[:],
                    attnTs[i][:, kk * P : (kk + 1) * P],
                    v_sb[:, kb0 + kk, :],
                    start=(kk == 0),
                    stop=(kk == nkb - 1),
                )
            o_t = p_at.tile([P, D], BF16, tag="o_t", bufs=3, name=f"o_t{i}")
            nc.vector.tensor_scalar_mul(out=o_t[:], in0=ps_o[:], scalar1=recips[i][:, 0:1])
            o_ts[i] = o_t

        def w2_c(i):  # oT transposes + fp8 split eviction
            pt2 = psum_t.tile([P, 512], BF16, tag="ptb", name=f"otp{i}")
            for m in range(DC):
                nc.tensor.transpose(
                    out=pt2[:, m * P : (m + 1) * P],
                    in_=o_ts[i][:, m * P : (m + 1) * P],
                    identity=identb[:],
                )
            oslc = oT8[:, :, i * P : (i + 1) * P]
            nc.scalar.activation(out=oslc, in_=pt2[:], func=AF.Copy, bias=0.0, scale=HS)
            nc.vector.scalar_tensor_tensor(
                out=oT8b[:, :, i * P : (i + 1) * P], in0=pt2[:], scalar=HS, in1=oslc,
                op0=mybir.AluOpType.mult, op1=mybir.AluOpType.subtract,
            )


        # t-major order: all groups needing hT[0:512] first (PE is in-order)
        for t in range(SEQ // 512):
            for (wt, bt, dst) in ((wq_sb, bq_sb, qT), (wk_sb, bk_sb, kT)):
                for m in range(DC):
                    ps = psum.tile([P, 512], F32, tag="ps")
                    i_mm = 0
                    for s_h, s_w in TERMS:
                        hsrc = hT8 if s_h == 0 else hT8b
                        for k0 in (0, 2):
                            nc.tensor.matmul(
                                ps[:],
                                wt[:, s_w, k0 : k0 + 2, m * P : (m + 1) * P],
                                hsrc[:, k0 : k0 + 2, t * 512 : (t + 1) * 512],
                                start=(i_mm == 0),
                                stop=(i_mm == 5),
                                perf_mode=DR,
                            )
                            i_mm += 1
                    dslc = dst[:, m, t * 512 : (t + 1) * 512]
                    if bt is not None:
                        nc.scalar.activation(
                            out=dslc, in_=ps[:], func=AF.Identity,
                            bias=bt[:, m : m + 1], scale=OSCALE,
                        )
                    elif dst is kT:
                        nc.vector.tensor_scalar_mul(out=dslc, in0=ps[:], scalar1=OSCALE)
                    else:
                        nc.scalar.activation(
                            out=dslc, in_=ps[:], func=AF.Copy, bias=0.0, scale=OSCALE,
                        )
            for j in range(4 * t, 4 * t + 4):
                ps = psum.tile([P, 512], F32, tag="ps")
                i_mm = 0
                for s_h, s_w in TERMS:
                    hsrc = hT8 if s_h == 0 else hT8b
                    for k0 in (0, 2):
                        nc.tensor.matmul(
                            ps[:],
                            hsrc[:, k0 : k0 + 2, j * P : (j + 1) * P],
                            wv_sb[:, s_w, k0 : k0 + 2, :],
                            start=(i_mm == 0),
                            stop=(i_mm == 5),
                            perf_mode=DR,
                        )
                        i_mm += 1
                if bv_bc is not None:
                    nc.vector.scalar_tensor_tensor(
                        out=v_sb[:, j, :], in0=ps[:], scalar=OSCALE, in1=bv_bc[:],
                        op0=mybir.AluOpType.mult, op1=mybir.AluOpType.add,
                    )
                else:
                    nc.vector.tensor_scalar_mul(out=v_sb[:, j, :], in0=ps[:], scalar1=OSCALE)
            # scores for query chunks whose full k-window is now resident;
            # second half also ramps the attention pipeline (slots 0..3)
            for i in range(4 * t, 4 * t + 4):
                scores_i(i)
                if t == 1:
                    k_ = i - 4
                    w2_a(k_)
                    if k_ >= 1:
                        w2_b(k_ - 1)
                    if k_ >= 3:
                        w2_c(k_ - 3)

        p_wq.release()
        p_qk.release()

        # ----- right-side pools for FFN phase -----
        whpool = tc.alloc_tile_pool(name="whstream", bufs=wh_bufs, side="right")
        p_h1 = tc.alloc_tile_pool(name="h1p", bufs=1, side="right")
        h1_sb = p_h1.tile([P, NT, D], BF16, tag="h1")
        h1T8 = p_h1.tile([P, DC, SEQ], F8, tag="h1T8")
        h1T8b = p_h1.tile([P, DC, SEQ], F8, tag="h1T8b")
        w1_sb = p_h1.tile([P, 2, DC, DFF], F8, tag="w1")
        nc.sync.dma_start(out=w1_sb[:], in_=w1_d[:].rearrange("s (k p) o -> p s k o", p=P))

        # ---------- stage 3 wave 2 + stage 4, software-pipelined ----------
        p_st4 = tc.alloc_tile_pool(name="st4", bufs=3)
        def s4_proj(j):  # attn projection + residual + LN1 (no transpose)
            ps = psum.tile([P, 512], F32, tag="ps", name=f"psp{j}")
            i_mm = 0
            for s_o, s_w in TERMS:
                osrc = oT8 if s_o == 0 else oT8b
                for k0 in (0, 2):
                    nc.tensor.matmul(
                        ps[:],
                        osrc[:, k0 : k0 + 2, j * P : (j + 1) * P],
                        wo_sb[:, s_w, k0 : k0 + 2, :],
                        start=(i_mm == 0),
                        stop=(i_mm == 5),
                        perf_mode=DR,
                    )
                    i_mm += 1
            r1 = p_st4.tile([P, D], F32, tag="r1", name=f"r1_{j}")
            nc.vector.scalar_tensor_tensor(
                out=r1[:], in0=ps[:], scalar=OSCALE, in1=h_sb[:, j, :],
                op0=mybir.AluOpType.mult, op1=mybir.AluOpType.add,
            )
            if bo_bc is not None:
                nc.vector.tensor_add(out=r1[:], in0=r1[:], in1=bo_bc[:])
            stats = small.tile([P, 6], F32, tag="stats")
            nc.vector.bn_stats(out=stats[:], in_=r1[:])
            mv = small.tile([P, 2], F32, tag="mv")
            nc.vector.bn_aggr(out=mv[:], in_=stats[:])
            stdt = small.tile([P, 1], F32, tag="stdt")
            nc.scalar.activation(
                out=stdt[:], in_=mv[:, 1:2], func=AF.Sqrt,
                bias=eps_t[:, 0:1], scale=1.0,
            )
            rstd = small.tile([P, 1], F32, tag="rstd")
            nc.vector.reciprocal(out=rstd[:], in_=stdt[:])
            nc.vector.tensor_scalar(
                out=h1_sb[:, j, :], in0=r1[:],
                scalar1=mv[:, 0:1], scalar2=rstd[:, 0:1],
                op0=mybir.AluOpType.subtract, op1=mybir.AluOpType.mult,
            )
            if g1_bc is not None:
                nc.vector.tensor_mul(out=h1_sb[:, j, :], in0=h1_sb[:, j, :], in1=g1_bc[:])
            if be1_bc is not None:
                nc.vector.tensor_add(out=h1_sb[:, j, :], in0=h1_sb[:, j, :], in1=be1_bc[:])

        def s4_trans(j):  # h1 transposes + fp8 split eviction
            pt3 = psum_t.tile([P, 512], BF16, tag="ptb", name=f"h1p{j}")
            for m in range(DC):
                nc.tensor.transpose(
                    out=pt3[:, m * P : (m + 1) * P],
                    in_=h1_sb[:, j, m * P : (m + 1) * P],
                    identity=identb[:],
                )
            h1slc = h1T8[:, :, j * P : (j + 1) * P]
            nc.scalar.activation(out=h1slc, in_=pt3[:], func=AF.Copy, bias=0.0, scale=HS)
            nc.vector.scalar_tensor_tensor(
                out=h1T8b[:, :, j * P : (j + 1) * P], in0=pt3[:], scalar=HS, in1=h1slc,
                op0=mybir.AluOpType.mult, op1=mybir.AluOpType.subtract,
            )

        # deeper pipeline: 2 slots of slack before the projection (oT fp8
        # quant chain) and before the h1 transpose (LN1 chain)
        p_w12 = tc.alloc_tile_pool(name="w12", bufs=1, side="right")
        w2_sb = p_w12.tile([P, 2, FC, D], F8, tag="w2")
        nc.sync.dma_start(out=w2_sb[:], in_=w2_d[:].rearrange("s (k p) o -> p s k o", p=P))

        # ---------- stage 5: FFN up, f1T = relu(W1^T @ h1T + b1) ----------
        p_f1 = tc.alloc_tile_pool(name="f1p", bufs=1, side="right")
        f1T8 = p_f1.tile([P, FC, SEQ], F8, tag="f1T8")
        f1T8b = p_f1.tile([P, FC, SEQ], F8, tag="f1T8b")
        def ffn1_group(n, t):
            ps = psum.tile([P, 512], F32, tag="ps", name=f"psf{n}_{t}")
            i_mm = 0
            for s_h, s_w in TERMS:
                hsrc = h1T8 if s_h == 0 else h1T8b
                for k0 in (0, 2):
                    nc.tensor.matmul(
                        ps[:],
                        w1_sb[:, s_w, k0 : k0 + 2, n * P : (n + 1) * P],
                        hsrc[:, k0 : k0 + 2, t * 512 : (t + 1) * 512],
                        start=(i_mm == 0),
                        stop=(i_mm == 5),
                        perf_mode=DR,
                    )
                    i_mm += 1
            # f1 = relu(psum*OSCALE + b1), stored as fp8 hi+lo at scale HS
            # (b1 is host-prescaled by HS, activation scale folds HS*OSCALE)
            ftmp = p_f1.tile([P, 512], F32, tag="ftmp", bufs=2, name=f"ftmp{n}_{t}")
            nc.scalar.activation(
                out=ftmp[:], in_=ps[:], func=AF.Relu,
                bias=(b1_sb[:, n : n + 1] if b1_sb is not None else 0.0),
                scale=HS * OSCALE,
            )
            fslc = f1T8[:, n, t * 512 : (t + 1) * 512]
            nc.vector.tensor_copy(out=fslc, in_=ftmp[:])
            nc.gpsimd.tensor_sub(
                out=f1T8b[:, n, t * 512 : (t + 1) * 512], in0=ftmp[:], in1=fslc,
            )


        for k in range(4, NT + 7):
            if k < NT:
                w2_a(k)
            if 1 <= k < NT + 1:
                w2_b(k - 1)
            if 3 <= k < NT + 3:
                w2_c(k - 3)
            if 5 <= k < NT + 5:
                s4_proj(k - 5)
            if 7 <= k < NT + 7:
                s4_trans(k - 7)
            if 11 <= k:
                # fill the pipeline tail with FFN-up work on the first half
                ffn1_group(2 * (k - 11), 0)
                ffn1_group(2 * (k - 11) + 1, 0)

        p_st4.release()
        p_at.release()
        p_v.release()
        p_oT.release()
        p_woh.release()

        # ---------- stage 6: FFN down + residual + LN2 (pipelined) ----------
        def s6_main(j):
            ps = psum.tile([P, 512], F32, tag="ps", name=f"ps6_{j}")
            i_mm = 0
            for s_f, s_w in TERMS:
                fsrc = f1T8 if s_f == 0 else f1T8b
                for n0 in (0, 2, 4, 6):
                    nc.tensor.matmul(
                        ps[:],
                        fsrc[:, n0 : n0 + 2, j * P : (j + 1) * P],
                        w2_sb[:, s_w, n0 : n0 + 2, :],
                        start=(i_mm == 0),
                        stop=(i_mm == 11),
                        perf_mode=DR,
                    )
                    i_mm += 1
            r2 = p_f1.tile([P, D], F32, tag="r2", bufs=3, name=f"r2_{j}")
            nc.vector.scalar_tensor_tensor(
                out=r2[:], in0=ps[:], scalar=OSCALE, in1=h1_sb[:, j, :],
                op0=mybir.AluOpType.mult, op1=mybir.AluOpType.add,
            )
            if b2_bc is not None:
                nc.vector.tensor_add(out=r2[:], in0=r2[:], in1=b2_bc[:])
            stats = small.tile([P, 6], F32, tag="stats")
            nc.vector.bn_stats(out=stats[:], in_=r2[:])
            mv = small.tile([P, 2], F32, tag="mv")
            nc.vector.bn_aggr(out=mv[:], in_=stats[:])
            stdt = small.tile([P, 1], F32, tag="stdt")
            # sqrt((var + eps)/HS^2) = sqrt(var+eps)/HS, so rstd = HS/std:
            # h2_t holds HS * LN(r2), pre-scaled for the fp8 head.
            nc.scalar.activation(
                out=stdt[:], in_=mv[:, 1:2], func=AF.Sqrt,
                bias=eps_hs[:, 0:1], scale=1.0 / (HS * HS),
            )
            rstd = small.tile([P, 1], F32, tag="rstd")
            nc.vector.reciprocal(out=rstd[:], in_=stdt[:])
            h2_t = p_f1.tile([P, D], BF16, tag="h2_t", bufs=3, name=f"h2t_{j}")
            nc.vector.tensor_scalar(
                out=h2_t[:], in0=r2[:],
                scalar1=mv[:, 0:1], scalar2=rstd[:, 0:1],
                op0=mybir.AluOpType.subtract, op1=mybir.AluOpType.mult,
            )
            if g2_bc is not None:
                nc.vector.tensor_mul(out=h2_t[:], in0=h2_t[:], in1=g2_bc[:])
            if be2_bc is not None:
                nc.vector.tensor_add(out=h2_t[:], in0=h2_t[:], in1=be2_bc[:])
            return h2_t

        h2ts = [None] * NT

        def s6_trans(j):
            pt = psum_t.tile([P, 512], BF16, tag="ptb", name=f"h2p{j}")
            for m in range(DC):
                nc.tensor.transpose(
                    out=pt[:, m * P : (m + 1) * P],
                    in_=h2ts[j][:, m * P : (m + 1) * P],
                    identity=identb[:],
                )
            # pt holds HS*h2T; quantize to fp8 + same-scale fp8 residual
            nc.scalar.copy(out=h8T[j][:, :, :], in_=pt[:])
            nc.vector.tensor_sub(out=h8bT[j][:, :, :], in0=pt[:], in1=h8T[j][:, :, :])

        # head chunks for vc=0..3 interleaved into stage-6 so PE fills LN waits
        wh_rr = wh_d[:].rearrange("(k p) v -> p k v", p=P)
        whb_rr = whb_d[:].rearrange("(k p) v -> p k v", p=P)
        _bh_tiles = {}

        def load_whv(vc, name):
            wv = whpool.tile([P, DC, 512], F8, tag="whv", name=f"whv{name}")
            nc.sync.dma_start(out=wv[:], in_=wh_rr[:, :, vc * 512 : (vc + 1) * 512])
            wvb = whpool.tile([P, DC, 512], F8, tag="whvb", name=f"whvb{name}")
            nc.sync.dma_start(out=wvb[:], in_=whb_rr[:, :, vc * 512 : (vc + 1) * 512])
            if bh_d is not None:
                bh_bc = whpool.tile([P, 512], F32, tag="bh", bufs=8, name=f"bh{name}")
                nc.sync.dma_start(
                    out=bh_bc[:], in_=_bcast_ap(bh_d[vc * 512 : (vc + 1) * 512])
                )
                _bh_tiles[vc] = bh_bc
            return wv, wvb

        N_IL = 4  # head vocab chunks interleaved into the stage-6 loop
        whvs = {}
        otiles = {}
        for vc in range(2):
            whvs[vc] = load_whv(vc, str(vc))
            otiles[vc] = opool.tile(
                [P, NT, 512], BF16 if OUT_BF16 else F32, tag="ot", name=f"otile{vc}"
            )

        def head_j(whv, otile, j, toggle):
            wv, wvb = whv
            ps = psum.tile([P, 512], F32, tag="ps", name=f"psh{toggle}_{j}")
            terms = ((h8T[j], wv), (h8T[j], wvb), (h8bT[j], wv))
            n_mm = len(terms) * 2
            i_mm = 0
            for lh, rh in terms:
                for k0 in (0, 2):
                    nc.tensor.matmul(
                        ps[:],
                        lh[:, k0 : k0 + 2, :],
                        rh[:, k0 : k0 + 2, :],
                        start=(i_mm == 0),
                        stop=(i_mm == n_mm - 1),
                        perf_mode=DR,
                    )
                    i_mm += 1
            if bh_sb_for(toggle) is not None:
                nc.vector.scalar_tensor_tensor(
                    out=otile[:, j, :], in0=ps[:], scalar=OSCALE,
                    in1=bh_sb_for(toggle)[:],
                    op0=mybir.AluOpType.mult, op1=mybir.AluOpType.add,
                )
            elif j % 2 == 0:
                nc.vector.tensor_scalar_mul(out=otile[:, j, :], in0=ps[:], scalar1=OSCALE)
            else:
                nc.scalar.activation(
                    out=otile[:, j, :], in_=ps[:], func=AF.Copy,
                    bias=0.0, scale=OSCALE,
                )

        def bh_sb_for(key):
            return _bh_tiles.get(key)

        for n in range(FC):
            ffn1_group(n, 1)
            if n % 2 == 1:
                h2ts[n // 2] = s6_main(n // 2)

        out_r = out_d[:].rearrange("(j p) v -> p j v", p=P)

        def out_dma(vc, otile, half):
            sl = slice(0, 4) if half == 0 else slice(4, 8)
            nc.sync.dma_start(
                out=out_r[:, sl, vc * 512 : (vc + 1) * 512], in_=otile[:, sl, :]
            )

        for k in range(NT + N_IL + 3):
            if 4 <= k < NT:
                h2ts[k] = s6_main(k)
            if 2 <= k <= NT + 1:
                s6_trans(k - 2)
            if k == 0:
                whvs[2] = load_whv(2, "2")
                otiles[2] = opool.tile(
                    [P, NT, 512], BF16 if OUT_BF16 else F32, tag="ot", name="otile2"
                )
            if k == 2:
                whvs[3] = load_whv(3, "3")
                otiles[3] = opool.tile(
                    [P, NT, 512], BF16 if OUT_BF16 else F32, tag="ot", name="otile3"
                )
            if k == 4:
                whvs[4] = load_whv(4, "4")
            if k == 6:
                whvs[5] = load_whv(5, "5")
            for vcix in range(N_IL):
                kk = k - 3 - vcix
                if 0 <= kk < NT:
                    head_j(whvs[vcix], otiles[vcix], kk, vcix)
                    if kk == 3:
                        out_dma(vcix, otiles[vcix], 0)
                    elif kk == NT - 1:
                        out_dma(vcix, otiles[vcix], 1)

        p_f1.release()
        p_w12.release()
        p_h1.release()

        # ---------- stage 7: vocab head (vc >= N_IL) ----------
        for vc in range(N_IL, NV):
            whv = whvs.get(vc) or load_whv(vc, str(vc))
            otile = opool.tile([P, NT, 512], BF16 if OUT_BF16 else F32, tag="ot")
            last = vc >= NV - 2
            for j in range(NT):
                head_j(whv, otile, j, vc)
                if j == 3:
                    out_dma(vc, otile, 0)
                elif last and j >= 4:
                    # last chunk: per-j tail so the final DMA is tiny
                    nc.sync.dma_start(
                        out=out_r[:, j, vc * 512 : (vc + 1) * 512],
                        in_=otile[:, j, :],
                    )
            if not last:
                out_dma(vc, otile, 1)

        whpool.release()
        p_h2T.release()
        opool.release()
        psum_t.release()
        psum.release()
        small.release()
        const.release()

    nc.finalize()
    return nc


_PROGRAM_CACHE: dict = {}


def _get_program(flags: dict) -> bass.Bass:
    key = tuple(sorted(flags.items()))
    if key not in _PROGRAM_CACHE:
        _PROGRAM_CACHE[key] = _build_program(flags)
    return _PROGRAM_CACHE[key]


def _prep(x, embed_tab, row_embed, col_embed, Wq, bq, Wk, bk, Wv, bv, Wo, bo,
          ln1_g, ln1_b, W1, b1, W2, b2, ln2_g, ln2_b, Wh, bh):
    """Shared host-side prep: flags, common input map, per-core x shards."""
    f32c = lambda a: np.ascontiguousarray(np.asarray(a, dtype=np.float32))
    x = np.asarray(x)
    B = x.shape[0]
    assert x.shape == (B, SEQ)

    import ml_dtypes
    bf16c = lambda a: np.ascontiguousarray(
        np.asarray(a, dtype=np.float32).astype(ml_dtypes.bfloat16)
    )

    def split8(w):
        ws = f32c(w) * np.float32(WS)
        hi = ws.astype(ml_dtypes.float8_e4m3)
        lo = (ws - hi.astype(np.float32)).astype(ml_dtypes.float8_e4m3)
        return np.ascontiguousarray(np.stack([hi, lo], axis=0))

    arrs = dict(
        wq=split8(Wq), wk=split8(Wk), wv=split8(Wv),
        wo=split8(Wo), w1=split8(W1), w2=split8(W2),
        identb=np.eye(P, dtype=np.float32).astype(ml_dtypes.bfloat16),
    )
    w512 = f32c(Wh) * np.float32(WS)
    w8 = w512.astype(ml_dtypes.float8_e4m3)
    w8b = (w512 - w8.astype(np.float32)).astype(ml_dtypes.float8_e4m3)
    arrs["wh"] = np.ascontiguousarray(w8)
    arrs["whb"] = np.ascontiguousarray(w8b)
    # input prep: embedding rows gathered + positional encoding added on host,
    # shipped per-core as a dense bf16 [SEQ, D] activation
    pos = np.concatenate(
        [np.repeat(f32c(row_embed), GW, axis=0), np.tile(f32c(col_embed), (GH, 1))],
        axis=-1,
    ).astype(np.float32)
    emb32 = f32c(embed_tab)
    hxs = [bf16c(emb32[x[c]] + pos) for c in range(B)]
    arrs["maskt"] = _mask_tiles().astype(ml_dtypes.bfloat16)

    # b1 feeds the HS-prescaled relu; be2 the HS-prescaled LN2 output
    bias_map = dict(
        bq=f32c(bq), bk=f32c(bk), bv=f32c(bv), bo=f32c(bo),
        b1=f32c(b1) * np.float32(HS),
        b2=f32c(b2), bh=f32c(bh), be1=f32c(ln1_b), be2=f32c(ln2_b) * np.float32(HS),
    )
    gain_map = dict(g1=f32c(ln1_g), g2=f32c(ln2_g))
    flags = {k: bool(np.any(v)) for k, v in bias_map.items()}
    flags.update({k: bool(np.any(v != 1.0)) for k, v in gain_map.items()})
    for k, v in {**bias_map, **gain_map}.items():
        if flags[k]:
            arrs[k] = v

    return flags, arrs, hxs, B


def kernel(**inputs):
    flags, arrs, hxs, B = _prep(**inputs)
    nc = _get_program(flags)
    core_ids = list(range(8))
    in_maps = [{**arrs, "hx": hxs[c % B]} for c in core_ids]
    res = run_bass_kernel_spmd(nc, in_maps, core_ids)
    out = np.stack([res.results[c]["out"] for c in range(B)], axis=0)
    return np.asarray(out, dtype=np.float32)

